# revision 19
# baseline (speedup 1.0000x reference)
"""Paged prefill attention (sparse_attention) on 8 Trainium2 NeuronCores.

Problem (hardcoded, mirrors the reference):
  q:        [2048, 32, 128] f32   (2 seqs x 1024 query tokens, 32 heads)
  k_cache:  [64, 64, 8, 128] f32  (64 physical blocks x 64 tokens x 8 kv heads)
  v_cache:  [64, 64, 8, 128] f32
  cu_seqlens_q: [0, 1024, 2048]
  cu_seqlens_k: [0, 2048, 4096]
  block_tables: [2, 32] int32 permutation of the 64 physical blocks
  out:      [2048, 32, 128] f32

Sharding: tensor-parallel by kv head. Core h gets kv head h plus its 4
query heads (GQA group 4), both full sequences. Each core runs the same
program (SPMD); the block-table gather is baked into the DMA descriptors
(the table is shared across heads, so one program serves all cores).

Per-core algorithm (S^T layout flash attention, fp16 matmuls):
  - K blocks are DMA-gathered per the block table, transposed on the PE
    (fp32), and stored as kT [d=128, tok] fp16.
  - Q tiles likewise transposed to qT [d=128, tok] fp16.
  - V chunks ([128 tok, 128 d]) are cast to fp16 with a ones column
    appended -> vP [128, 129] per chunk.
  - QK: S^T[k,q] = kT_tile.T @ qT, per 128-k-tile x 512-q-chunk, into
    PSUM, skipping fully-masked chunks (causal + 1024 history).
  - diagonal 128x128 tiles get an additive -1e10 upper-triangular mask.
  - exp(scale*s) on ScalarE straight from PSUM into an fp16 S^T buffer.
  - PV: for each 128-q tile, accumulate over k chunks
    out[q, 0:129] += expS_chunk.T @ vP_chunk  -- col 128 is the softmax
    denominator (ones column), cols 0:128 the unnormalized output.
  - normalize with VectorE reciprocal + per-partition scalar multiply,
    DMA out.
"""

import numpy as np

NUM_SEQS = 2
LQ = 1024
HIST = 1024
LK = LQ + HIST
NUM_HEADS = 32
NUM_KV_HEADS = 8
GROUP = NUM_HEADS // NUM_KV_HEADS  # 4 q heads per kv head / core
HEAD_DIM = 128
BLOCK_SIZE = 64
NBLK = LK // BLOCK_SIZE        # 32 logical blocks per sequence
TOTAL_BLOCKS = NUM_SEQS * NBLK  # 64 physical blocks
NCH = LK // 128                 # 16 128-token kv chunks per sequence
NQT = LQ // 128                 # 8 128-token q tiles per sequence
SCALE = 1.0 / float(np.sqrt(HEAD_DIM))
NEG = -1e10

_CACHE = {}


def _build_program(bt: np.ndarray):
    from contextlib import ExitStack

    import concourse.bass as bass
    import concourse.mybir as mybir
    import concourse.tile as tile
    from concourse import bacc
    from concourse.masks import make_identity

    f32 = mybir.dt.float32
    f16 = mybir.dt.float16

    nc = bacc.Bacc()
    q_d = nc.dram_tensor("q", [NUM_SEQS * LQ, GROUP, HEAD_DIM], f32,
                         kind="ExternalInput")
    k_d = nc.dram_tensor("k", [TOTAL_BLOCKS, BLOCK_SIZE, HEAD_DIM], f32,
                         kind="ExternalInput")
    v_d = nc.dram_tensor("v", [TOTAL_BLOCKS, BLOCK_SIZE, HEAD_DIM], f32,
                         kind="ExternalInput")
    o_d = nc.dram_tensor("out", [NUM_SEQS * LQ, GROUP, HEAD_DIM], f32,
                         kind="ExternalOutput")

    with tile.TileContext(nc) as tc, ExitStack() as ctx:
        consts = ctx.enter_context(tc.tile_pool(name="consts", bufs=1))
        persist = ctx.enter_context(tc.tile_pool(name="persist", bufs=1))
        stage = ctx.enter_context(tc.tile_pool(name="stage", bufs=4))
        small = ctx.enter_context(tc.tile_pool(name="small", bufs=4))
        es_pool = ctx.enter_context(tc.tile_pool(name="es", bufs=2))
        tp_ps = ctx.enter_context(tc.tile_pool(name="tp_ps", bufs=2, space="PSUM"))
        sc_ps = ctx.enter_context(tc.tile_pool(name="sc_ps", bufs=2, space="PSUM"))
        oc_ps = ctx.enter_context(tc.tile_pool(name="oc_ps", bufs=2, space="PSUM"))

        ident = consts.tile([128, 128], f32, tag="ident")
        make_identity(nc, ident[:, :])

        cmask = consts.tile([128, 128], f32, tag="cmask")
        nc.gpsimd.memset(cmask[:, :], 0.0)
        # keep (pass 0) where q_col >= k_row, else fill NEG
        nc.gpsimd.affine_select(
            out=cmask[:, :], in_=cmask[:, :],
            compare_op=mybir.AluOpType.is_ge, fill=NEG,
            base=0, pattern=[[1, 128]], channel_multiplier=-1,
        )

        qT = persist.tile([128, NUM_SEQS * GROUP * LQ], f16, tag="qT")
        kT = persist.tile([128, NUM_SEQS * LK], f16, tag="kT")
        vP = persist.tile([128, NUM_SEQS * NCH * 129], f16, tag="vP")

        # ---- K / V load, gather, transpose (K), cast ----
        for s in range(NUM_SEQS):
            for c in range(NCH):  # chunk c = logical blocks 2c, 2c+1
                kst = stage.tile([128, 128], f32, tag="kst")
                vst = stage.tile([128, 128], f32, tag="vst")
                for half in range(2):
                    phys = int(bt[s, 2 * c + half])
                    nc.sync.dma_start(
                        out=kst[half * 64:(half + 1) * 64, :],
                        in_=k_d[phys, :, :])
                    nc.sync.dma_start(
                        out=vst[half * 64:(half + 1) * 64, :],
                        in_=v_d[phys, :, :])
                pst = tp_ps.tile([128, 128], f32, tag="tp")
                nc.tensor.transpose(pst[:, :], kst[:, :], ident[:, :])
                nc.vector.tensor_copy(
                    kT[:, s * LK + c * 128:s * LK + (c + 1) * 128], pst[:, :])
                base = (s * NCH + c) * 129
                nc.vector.tensor_copy(vP[:, base:base + 128], vst[:, :])
                nc.vector.memset(vP[:, base + 128:base + 129], 1.0)

        # ---- Q load + transpose ----
        for s in range(NUM_SEQS):
            for h in range(GROUP):
                qbase = (s * GROUP + h) * LQ
                for qt in range(NQT):
                    qst = stage.tile([128, 128], f32, tag="qst")
                    nc.sync.dma_start(
                        out=qst[:, :],
                        in_=q_d[s * LQ + qt * 128:s * LQ + (qt + 1) * 128, h, :])
                    pst = tp_ps.tile([128, 128], f32, tag="tp")
                    nc.tensor.transpose(pst[:, :], qst[:, :], ident[:, :])
                    nc.vector.tensor_copy(
                        qT[:, qbase + qt * 128:qbase + (qt + 1) * 128],
                        pst[:, :])

        # ---- attention per (seq, head) ----
        for s in range(NUM_SEQS):
            for h in range(GROUP):
                qbase = (s * GROUP + h) * LQ
                es = es_pool.tile([128, NCH * LQ], f16, tag="es")
                for kt in range(NCH):
                    q_lo = max(0, (kt - NCH // 2) * 128)
                    qc0 = (q_lo // 512) * 512
                    width = LQ - qc0
                    ps = sc_ps.tile([128, 1024], f32, tag="sc")
                    for j in range(width // 512):
                        qc = qc0 + j * 512
                        nc.tensor.matmul(
                            ps[:, j * 512:(j + 1) * 512],
                            kT[:, s * LK + kt * 128:s * LK + (kt + 1) * 128],
                            qT[:, qbase + qc:qbase + qc + 512],
                            start=True, stop=True)
                    if kt >= NCH // 2:
                        qd = kt - NCH // 2
                        off = qd * 128 - qc0
                        nc.vector.tensor_add(
                            ps[:, off:off + 128], ps[:, off:off + 128],
                            cmask[:, :])
                    nc.scalar.activation(
                        es[:, kt * LQ + qc0:(kt + 1) * LQ],
                        ps[:, 0:width],
                        mybir.ActivationFunctionType.Exp, scale=SCALE)
                for qt in range(NQT):
                    nch_q = NCH // 2 + 1 + qt  # kv chunks 0 .. 8+qt
                    po = oc_ps.tile([128, 129], f32, tag="oc")
                    for c in range(nch_q):
                        nc.tensor.matmul(
                            po[:, :],
                            es[:, c * LQ + qt * 128:c * LQ + (qt + 1) * 128],
                            vP[:, (s * NCH + c) * 129:(s * NCH + c + 1) * 129],
                            start=(c == 0), stop=(c == nch_q - 1))
                    rc = small.tile([128, 1], f32, tag="rc")
                    nc.vector.reciprocal(rc[:, :], po[:, 128:129])
                    ob = small.tile([128, 128], f32, tag="ob")
                    nc.vector.tensor_scalar_mul(ob[:, :], po[:, 0:128], rc[:, :])
                    nc.sync.dma_start(
                        out=o_d[s * LQ + qt * 128:s * LQ + (qt + 1) * 128, h, :],
                        in_=ob[:, :])

    nc.compile()
    return nc


def _get_program(bt: np.ndarray):
    key = bt.tobytes()
    if key not in _CACHE:
        _CACHE[key] = _build_program(bt)
    return _CACHE[key]


def kernel(q, k_cache, v_cache, cu_seqlens_q, cu_seqlens_k, block_tables,
           _want_trace=False):
    from concourse import bass_utils

    q = np.ascontiguousarray(np.asarray(q, dtype=np.float32))
    k_cache = np.ascontiguousarray(np.asarray(k_cache, dtype=np.float32))
    v_cache = np.ascontiguousarray(np.asarray(v_cache, dtype=np.float32))
    bt = np.asarray(block_tables, dtype=np.int32)

    assert q.shape == (NUM_SEQS * LQ, NUM_HEADS, HEAD_DIM)
    assert k_cache.shape == (TOTAL_BLOCKS, BLOCK_SIZE, NUM_KV_HEADS, HEAD_DIM)
    assert v_cache.shape == (TOTAL_BLOCKS, BLOCK_SIZE, NUM_KV_HEADS, HEAD_DIM)
    assert bt.shape == (NUM_SEQS, NBLK)
    assert bt.min() >= 0

    nc = _get_program(bt)

    in_maps = []
    for core in range(NUM_KV_HEADS):
        in_maps.append({
            "q": np.ascontiguousarray(
                q[:, core * GROUP:(core + 1) * GROUP, :]),
            "k": np.ascontiguousarray(k_cache[:, :, core, :]),
            "v": np.ascontiguousarray(v_cache[:, :, core, :]),
        })

    res = bass_utils.run_bass_kernel_spmd(
        nc, in_maps, core_ids=list(range(NUM_KV_HEADS)),
        trace=_want_trace,
        **({"trace_cores": list(range(NUM_KV_HEADS)), "stitch_traces": True}
           if _want_trace else {}),
    )

    out = np.empty((NUM_SEQS * LQ, NUM_HEADS, HEAD_DIM), dtype=np.float32)
    for core in range(NUM_KV_HEADS):
        out[:, core * GROUP:(core + 1) * GROUP, :] = res.results[core]["out"]

    if _want_trace:
        return out, res
    return out
